# revision 14
# baseline (speedup 1.0000x reference)
"""NetVLAD pooling kernel for Trainium2 (8 NeuronCores, data-parallel over N).

Reference computation (per sample):
    xf   = l2norm(x, axis=C)                  # [C, P]
    Wn   = l2norm(W, axis=0)                  # [C, K]
    sal  = Wn^T @ xf                          # [K, P]   (sa_logits output)
    sa   = softmax(alpha * sal, axis=K)       # [K, P]
    vlad = sa @ xf^T - sa.sum(P) * W^T        # [K, C]
    vlad = l2norm(vlad, axis=C); flatten; l2norm

Device dataflow (per core, ns=8 samples):
  - x (fp32) loaded in [C, P] layout; bf16 copy of x loaded *transposed* via
    the DMA xbar into [P-chunk, C] layout ("slot" b holds pixels {32a+b}).
  - mm1: stationary = fp32 x slot-chunk, moving = Wn -> raw logits^T in PSUM
    ([pixels, K] layout, softmax axis = free dim).
  - r = 1/||x_p|| computed from bf16 x^T via fused square+reduce (DVE ttr),
    then ln/exp (single ACT table set for the whole kernel).
  - true logits = raw * r (ACT copy-with-scale from PSUM), exp on ACT,
    denominators via grouped DVE reduce, sa' = expo * (r/denom) in bf16.
  - vlad = accumulate over slots: sa'^T-chunk (stationary, bf16) x
    [x^T-chunk | ||x_p||] (moving, bf16) -> [K, C+1] PSUM; col C holds
    sum_p(sa) via the norm column (sa' * ||x|| = sa).
  - intra/global l2 norms via fused square-reduce + ln/exp; partition
    reduction + broadcast for the global norm via tiny matmuls.
  - sa_logits output: PE transpose of true-logits^T chunks back to [K, P].
"""

import sys

sys.path.insert(0, "/opt/trn_rl_repo")

import numpy as np
import ml_dtypes

import concourse.bass as bass
import concourse.bacc as bacc
import concourse.tile as tile
from concourse import mybir

F32 = mybir.dt.float32
BF16 = mybir.dt.bfloat16
AF = mybir.ActivationFunctionType
ALU = mybir.AluOpType

ALPHA = 50.0
N_CORES = 8
C = 128  # channels
K = 64   # clusters
CH = 128  # pixels per slot-chunk (partition dim of transposed layout)


def build_program(ns: int, P: int):
    """Build the per-core Bass program for ns samples of [C, P] pixels."""
    NCH = P // CH  # number of slot-chunks
    nc = bacc.Bacc()

    x_in = nc.declare_dram_parameter("x", [ns, C, P], F32, isOutput=False)
    xb_in = nc.declare_dram_parameter("xb", [ns, C, P], BF16, isOutput=False)
    w_in = nc.declare_dram_parameter("w", [C, K], F32, isOutput=False)
    id_in = nc.declare_dram_parameter("ident", [128, 128], F32, isOutput=False)
    oc_in = nc.declare_dram_parameter("ones_col", [128, 1], F32, isOutput=False)
    or_in = nc.declare_dram_parameter("ones_row", [1, K], F32, isOutput=False)
    vlad_out = nc.declare_dram_parameter("vlad", [ns, K * C], F32, isOutput=True)
    sal_out = nc.declare_dram_parameter("sal", [ns, K, P], F32, isOutput=True)

    with tile.TileContext(nc) as tc, \
         tc.tile_pool(name="const", bufs=1) as constp, \
         tc.tile_pool(name="xa", bufs=2) as xap, \
         tc.tile_pool(name="xtn", bufs=2) as xtnp, \
         tc.tile_pool(name="tlt", bufs=2) as tltp, \
         tc.tile_pool(name="xsq", bufs=2) as xsqp, \
         tc.tile_pool(name="expo", bufs=2) as expop, \
         tc.tile_pool(name="sap", bufs=2) as sapp, \
         tc.tile_pool(name="sala", bufs=2) as salap, \
         tc.tile_pool(name="small", bufs=3) as smallp, \
         tc.tile_pool(name="junk", bufs=1) as junkp, \
         tc.tile_pool(name="stat", bufs=1) as statp, \
         tc.tile_pool(name="mm1ps", bufs=2, space=bass.MemorySpace.PSUM) as mm1p, \
         tc.tile_pool(name="vladps", bufs=2, space=bass.MemorySpace.PSUM) as vladp, \
         tc.tile_pool(name="salps", bufs=2, space=bass.MemorySpace.PSUM) as salpp, \
         tc.tile_pool(name="tinyps", bufs=1, space=bass.MemorySpace.PSUM) as tinyp:

        # ---------------- constants / weight prep ----------------
        ident = constp.tile([128, 128], F32)
        nc.sync.dma_start(ident[:], id_in[:])
        ones_col = constp.tile([128, 1], F32)
        nc.sync.dma_start(ones_col[:], oc_in[:])
        ones_row = constp.tile([1, K], F32)
        nc.sync.dma_start(ones_row[:], or_in[:])
        w_sb = constp.tile([C, K], F32)
        nc.sync.dma_start(w_sb[:], w_in[:])

        # W^T via PE transpose
        wt_ps = tinyp.tile([K, C], F32, tag="tiny")
        nc.tensor.matmul(wt_ps[:], w_sb[:], ident[:], is_transpose=True)
        wt_sb = constp.tile([K, C], F32)
        nc.vector.tensor_copy(wt_sb[:], wt_ps[:])

        # column norms of W -> Wn^T = W^T * (1/||col||)
        wjunk = junkp.tile([K, C], F32, tag="wjunk")
        wnsq = constp.tile([K, 1], F32)
        nc.scalar.activation(wjunk[:], wt_sb[:], AF.Square, accum_out=wnsq[:])
        wln = constp.tile([K, 1], F32)
        nc.scalar.activation(wln[:], wnsq[:], AF.Ln)
        winv = constp.tile([K, 1], F32)
        nc.scalar.activation(winv[:], wln[:], AF.Exp, scale=-0.5)
        nc.vector.tensor_scalar_min(winv[:], winv[:], 1e12)
        wnt_sb = constp.tile([K, C], F32)
        nc.vector.tensor_scalar_mul(wnt_sb[:], wt_sb[:], winv[:])
        # Wn via transpose back
        wn_ps = tinyp.tile([C, K], F32, tag="tiny")
        nc.tensor.matmul(wn_ps[:], wnt_sb[:], ident[0:K, 0:K], is_transpose=True)
        wn_sb = constp.tile([C, K], F32)
        nc.vector.tensor_copy(wn_sb[:], wn_ps[:])

        # cross-sample tiles for the norm tails
        nsq_all = statp.tile([K, ns], F32)
        gsq_all = statp.tile([K, ns], F32)
        vlad2_all = statp.tile([K, ns * C], F32)
        vlad3_all = statp.tile([K, ns * C], F32)

        # ---------------- per-sample pipeline ----------------
        for s in range(ns):
            # load fp32 x [C, P]
            xa = xap.tile([C, P], F32, tag="xa")
            nc.sync.dma_start(xa[:], x_in[s])

            # transposed bf16 x: [128, NCH, CH+1]; col CH of each slot = ||x_p||
            xtn = xtnp.tile([128, NCH * (CH + 1)], BF16, tag="xtn")
            xtn3 = xtn[:].rearrange("p (n c) -> p n c", c=CH + 1)
            nc.sync.dma_start_transpose(xtn3[:, :, 0:CH], xb_in[s])

            # rsq per pixel: gpsimd squares (exact in fp32), grouped DVE reduce
            nc.gpsimd.memset(xtn3[:, :, CH], 0.0)
            xsq = xsqp.tile([128, NCH * (CH + 1)], F32, tag="xsq")
            nc.gpsimd.tensor_tensor(xsq[:], xtn[:], xtn[:], ALU.mult)
            xsq3 = xsq[:].rearrange("p (n c) -> p n c", c=CH + 1)
            rsqB = smallp.tile([128, NCH], F32, tag="rsqB")
            nc.vector.tensor_reduce(rsqB[:], xsq3[:, :, 0:CH],
                                    axis=mybir.AxisListType.X, op=ALU.add)
            # r = exp(-0.5*ln(rsq)); norm col = rsq * r
            lnB = smallp.tile([128, NCH], F32, tag="lnB")
            nc.scalar.activation(lnB[:], rsqB[:], AF.Ln)
            rB = smallp.tile([128, NCH], F32, tag="rB")
            nc.scalar.activation(rB[:], lnB[:], AF.Exp, scale=-0.5)
            # norm column written (bf16 cast) into xtn slot col CH
            nc.vector.tensor_tensor(xtn3[:, :, CH], rsqB[:], rB[:], ALU.mult)

            # mm1: raw logits^T per slot into PSUM groups of 8 slots
            xa3 = xa[:].rearrange("c (n a) -> c n a", a=CH)  # [C, NCH, 128]
            tlt = tltp.tile([128, NCH * K], F32, tag="tlt")
            expo = expop.tile([128, NCH * K], BF16, tag="expo")
            G = 8 if NCH % 8 == 0 else NCH
            for g in range(NCH // G):
                ps = mm1p.tile([128, G * K], F32, tag="mm1")
                for i in range(G):
                    b = g * G + i
                    nc.tensor.matmul(ps[:, i * K:(i + 1) * K], xa3[:, b, :],
                                     wn_sb[:], start=True, stop=True)
                for i in range(G):
                    b = g * G + i
                    # true logits = raw * r (ACT copy-with-scale from PSUM)
                    nc.scalar.activation(tlt[:, b * K:(b + 1) * K],
                                         ps[:, i * K:(i + 1) * K],
                                         AF.Copy, scale=rB[:, b:b + 1])
            # exp (batched, bf16 out)
            nc.scalar.activation(expo[:], tlt[:], AF.Exp, scale=ALPHA)
            # denominators (grouped reduce) and dinv' = r/denom
            denom = smallp.tile([128, NCH], F32, tag="denom")
            expo3 = expo[:].rearrange("p (n k) -> p n k", k=K)
            nc.vector.tensor_reduce(denom[:], expo3[:, :, :],
                                    axis=mybir.AxisListType.X, op=ALU.add)
            dinv = smallp.tile([128, NCH], F32, tag="dinv")
            nc.vector.reciprocal(dinv[:], denom[:])
            dinvp = smallp.tile([128, NCH], F32, tag="dinvp")
            nc.vector.tensor_tensor(dinvp[:], dinv[:], rB[:], ALU.mult)

            # sa' = expo * dinv' (bf16 4x)
            sap = sapp.tile([128, NCH * K], BF16, tag="sap")
            for b in range(NCH):
                nc.vector.tensor_scalar_mul(sap[:, b * K:(b + 1) * K],
                                            expo[:, b * K:(b + 1) * K],
                                            dinvp[:, b:b + 1])

            # vlad accumulation: [K, C+1] PSUM; col C = sum_p sa
            vps = vladp.tile([K, C + 1], F32, tag="vlad")
            for b in range(NCH):
                nc.tensor.matmul(vps[:], sap[:, b * K:(b + 1) * K],
                                 xtn3[:, b, :],
                                 start=(b == 0), stop=(b == NCH - 1))

            # vlad post: subtract ssum * W^T, intra-normalize (deferred tail)
            ssum = smallp.tile([K, 1], F32, tag="ssum")
            nc.vector.tensor_copy(ssum[:], vps[:, C:C + 1])
            wterm = smallp.tile([K, C], F32, tag="wterm")
            nc.vector.tensor_scalar_mul(wterm[:], wt_sb[:], ssum[:])
            vlad2 = vlad2_all[:, s * C:(s + 1) * C]
            nc.vector.tensor_tensor(vlad2, vps[:, 0:C], wterm[:], ALU.subtract)
            vjunk = junkp.tile([K, C], F32, tag="vjunk")
            nc.scalar.activation(vjunk[:], vlad2, AF.Square,
                                 accum_out=nsq_all[:, s:s + 1])

            # sa_logits output: PE transpose of true-logits^T chunks
            sala = salap.tile([K, P], F32, tag="sala")
            sala3 = sala[:].rearrange("k (n a) -> k n a", a=128)  # [K, NCH, 128]
            SG = 4 if NCH % 4 == 0 else 1
            for g in range(NCH // SG):
                sps = salpp.tile([K, SG * 128], F32, tag="salps")
                for i in range(SG):
                    b = g * SG + i
                    nc.tensor.matmul(sps[:, i * 128:(i + 1) * 128],
                                     tlt[:, b * K:(b + 1) * K], ident[:],
                                     is_transpose=True)
                sps3 = sps[:].rearrange("k (i a) -> k i a", a=128)
                if SG > 1:
                    nc.scalar.copy(sala3[:, g * SG:(g + 1) * SG, :], sps3[:])
                else:
                    nc.scalar.copy(sala3[:, g, :], sps3[:, 0, :])
            nc.sync.dma_start(sal_out[s], sala[:])

        # ---------------- norm tails (batched across samples) ----------------
        # intra-norm scale r2 = min(exp(-0.5 ln nsq), 1e12)
        nln = statp.tile([K, ns], F32)
        nc.scalar.activation(nln[:], nsq_all[:], AF.Ln)
        r2 = statp.tile([K, ns], F32)
        nc.scalar.activation(r2[:], nln[:], AF.Exp, scale=-0.5)
        nc.vector.tensor_scalar_min(r2[:], r2[:], 1e12)
        for s in range(ns):
            v3 = vlad3_all[:, s * C:(s + 1) * C]
            nc.vector.tensor_scalar_mul(v3, vlad2_all[:, s * C:(s + 1) * C],
                                        r2[:, s:s + 1])
            vjunk2 = junkp.tile([K, C], F32, tag="vjunk")
            nc.scalar.activation(vjunk2[:], v3, AF.Square,
                                 accum_out=gsq_all[:, s:s + 1])
        # global sumsq per sample: reduce over K partitions via PE
        gs_ps = tinyp.tile([1, ns], F32, tag="tiny")
        nc.tensor.matmul(gs_ps[:], ones_col[0:K, :], gsq_all[:])
        gs_sb = statp.tile([1, ns], F32)
        nc.vector.tensor_copy(gs_sb[:], gs_ps[:])
        gl = statp.tile([1, ns], F32)
        nc.scalar.activation(gl[:], gs_sb[:], AF.Ln)
        g_sb = statp.tile([1, ns], F32)
        nc.scalar.activation(g_sb[:], gl[:], AF.Exp, scale=-0.5)
        nc.vector.tensor_scalar_min(g_sb[:], g_sb[:], 1e12)
        # broadcast to K partitions
        gb_ps = tinyp.tile([K, ns], F32, tag="tiny")
        nc.tensor.matmul(gb_ps[:], ones_row[:], g_sb[:])
        gb_sb = statp.tile([K, ns], F32)
        nc.vector.tensor_copy(gb_sb[:], gb_ps[:])
        for s in range(ns):
            vout = smallp.tile([K, C], F32, tag="vout")
            nc.vector.tensor_scalar_mul(vout[:],
                                        vlad3_all[:, s * C:(s + 1) * C],
                                        gb_sb[:, s:s + 1])
            nc.sync.dma_start(vlad_out[s].rearrange("(k c) -> k c", c=C), vout[:])

    nc.compile()
    return nc


_PROGRAM_CACHE = {}


def get_program(ns, P):
    key = (ns, P)
    if key not in _PROGRAM_CACHE:
        _PROGRAM_CACHE[key] = build_program(ns, P)
    return _PROGRAM_CACHE[key]


def make_const_inputs():
    return {
        "ident": np.eye(128, dtype=np.float32),
        "ones_col": np.ones((128, 1), dtype=np.float32),
        "ones_row": np.ones((1, K), dtype=np.float32),
    }


def kernel(x, W=None, bias=None):
    out, _ = kernel_traced(x, W, bias, trace=False)
    return out


def kernel_traced(x, W=None, bias=None, trace=True, **trace_kwargs):
    from concourse.bass_utils import run_bass_kernel_spmd

    x = np.asarray(x)
    if W is None or bias is None:
        # Defensive: regenerate params exactly as reference.setup_inputs does.
        import jax
        key = jax.random.key(0)
        _, k2 = jax.random.split(key)
        if W is None:
            W = np.asarray(jax.random.normal(k2, (128, 64), dtype=np.float32)) \
                * (1.0 / np.sqrt(128.0))
        if bias is None:
            bias = np.zeros((64,), dtype=np.float32)
    W = np.asarray(W, dtype=np.float32)
    bias = np.asarray(bias, dtype=np.float32)
    assert np.all(bias == 0.0), "kernel assumes zero bias (as in setup_inputs)"

    N, Cx, H, Wsp = x.shape
    P = H * Wsp
    assert Cx == C and N % N_CORES == 0
    ns = N // N_CORES

    xf = np.ascontiguousarray(x.reshape(N, C, P).astype(np.float32))
    xb = np.ascontiguousarray(xf.astype(ml_dtypes.bfloat16))

    nc = get_program(ns, P)
    consts = make_const_inputs()
    in_maps = []
    for i in range(N_CORES):
        m = {
            "x": xf[i * ns:(i + 1) * ns],
            "xb": xb[i * ns:(i + 1) * ns],
            "w": W,
        }
        m.update(consts)
        in_maps.append(m)

    res = run_bass_kernel_spmd(nc, in_maps, list(range(N_CORES)),
                               trace=trace, **trace_kwargs)
    vlad = np.concatenate([r["vlad"] for r in res.results], axis=0)
    sal = np.concatenate([r["sal"] for r in res.results], axis=0)
    return (vlad, sal), res
